# revision 8
# baseline (speedup 1.0000x reference)
"""3x3 conv (256->256, stride 1, pad 1) as implicit GEMM on 8 TRN2 NeuronCores.

Data-parallel over batch: 32 images -> 4 per core; weight/bias replicated.

Per core, per image: x is resident in SBUF as two [128, 56, 58] channel
tiles (zero columns at w=0 and w=57 provide the horizontal conv padding,
keeping every tap a full 56-wide window). For each output row-tile of 8
rows and each of 2 output-channel tiles, 18 matmuls (9 conv taps x 2
input-channel tiles) accumulate into a PSUM tile [128, 8, 56]. Operands
are bf16 (1 cycle/row on the PE, same as fp32r, but LDWEIGHTS is 2x
faster via fast-weight-load so the stationary reload fully hides under
the ~187ns stream; fp32r's 187ns weight load was the critical path at
~210ns/matmul). Padding is handled by clipping each tap's row range via
3D access patterns; the center tap runs first with start=True so every
PSUM element's first write is an overwrite. Bias is fused into the
PSUM->SBUF eviction on the scalar engine (bf16 out, upcast on host).

Startup is choreographed so the PE starts ~8.5us in: the weight is
hosted in [c, co, tap', ci, o] order with the center tap first (tap'
order 4,0,1,2,3,5,6,7,8) and DMA'd in 3 pieces per co half on two
otherwise-idle rings (gpsimd: co0, vector: co1); x stages in 3 row
chunks so the first row-tile's data lands early; per-slice shadow-memory
dependency tracking lets the matmuls chase the arriving pieces. A few
dummy matmuls on zeros warm the PE clock gate (HAM) during the DMA wait
so the real matmuls run at 2.4 GHz from the start.
"""

from contextlib import ExitStack

import numpy as np

import os

import concourse.bass as bass  # noqa: F401  (bass types used via tc/nc)
import concourse.tile as tile
from concourse import bacc, mybir
from concourse.bass_utils import run_bass_kernel_spmd

N_CORES = 8
N_TOTAL = 32
N_PER = N_TOTAL // N_CORES  # 4 images per core
C = 256
H = W = 56
RT = 8          # output rows per PSUM tile -> 8*56 = 448 <= 512 (one bank)
NRT = H // RT   # 7 row tiles
F32 = mybir.dt.float32
# compute dtypes for the matmul operands (storage + PE streaming format)
_X_DT_NAME = os.environ.get("CONV_X_DTYPE", "bfloat16")
_W_DT_NAME = os.environ.get("CONV_W_DTYPE", "bfloat16")
X_DT = getattr(mybir.dt, _X_DT_NAME)
W_DT = getattr(mybir.dt, _W_DT_NAME)
X_NP = mybir.dt.np(X_DT)
W_NP = mybir.dt.np(W_DT)
N_DUMMY = int(os.environ.get("CONV_N_DUMMY", "6"))

# tap order in the hosted weight: center tap first so the first DMA piece
# carries the weights the first (start=True) matmuls need
WORDER = [4, 0, 1, 2, 3, 5, 6, 7, 8]
TIDX = {t: i for i, t in enumerate(WORDER)}
# x staging row chunks: row-tile rt reads rows 8rt-1..8rt+8, so chunk
# boundaries at 9/33 cover rt0 | rt1-3 | rt4-6 cumulatively
XCHUNKS = [(0, 9), (9, 33), (33, 56)]

_CACHE = {}


def _build():
    nc = bacc.Bacc(
        "TRN2", target_bir_lowering=False, debug=False, num_devices=N_CORES
    )
    xs = nc.dram_tensor(
        "xs", [N_PER, C, H, W], X_DT, kind="ExternalInput"
    ).ap()
    wt = nc.dram_tensor(
        "wt", [128, 2, 9, 2, 128], W_DT, kind="ExternalInput"
    ).ap()
    b2 = nc.dram_tensor("b2", [128, 2], F32, kind="ExternalInput").ap()
    out = nc.dram_tensor(
        "out", [N_PER, C, H, W], X_DT, kind="ExternalOutput"
    ).ap()

    # Accumulation order: center tap (full coverage) first so its
    # start=True write touches every element of the PSUM tile; then taps
    # in WORDER sequence (= weight DMA arrival order), both ci each.
    order = [(1, 1, 0), (1, 1, 1)]
    for t in WORDER[1:]:
        for ci in (0, 1):
            order.append((t // 3, t % 3, ci))

    with tile.TileContext(nc) as tc, ExitStack() as ctx:
        wpool = ctx.enter_context(tc.tile_pool(name="w", bufs=1))
        spool = ctx.enter_context(tc.tile_pool(name="s", bufs=2))
        bpool = ctx.enter_context(tc.tile_pool(name="b", bufs=1))
        xpool = ctx.enter_context(tc.tile_pool(name="x", bufs=1))
        opool = ctx.enter_context(tc.tile_pool(name="o", bufs=4))
        ppool = ctx.enter_context(tc.tile_pool(name="p", bufs=4, space="PSUM"))
        dpool = ctx.enter_context(tc.tile_pool(name="d", bufs=1, space="PSUM"))

        # PE warmup: a zero tile (memset early on the gpsimd queue) feeds
        # a few dummy matmuls into a scratch PSUM bank so the HAM clock
        # gate opens to 2.4 GHz while the real weight/x DMAs land.
        d_sb = bpool.tile([128, RT * W], X_DT)
        nc.gpsimd.memset(d_sb[:], 0.0)

        # weight DMA in tap pieces (center | next 4 | last 4) per co half
        # so the matmuls can start as soon as the early taps arrive. The
        # two center pieces ride at the head of the sync/scalar rings (in
        # front of the x chunks they unblock); the rest go on gpsimd.
        # Only sync/scalar/gpsimd rings can issue DMAs.
        w_sb = wpool.tile([128, 2, 9, 2, 128], W_DT)
        b_sb = bpool.tile([128, 2], F32)
        nc.gpsimd.dma_start(b_sb[:], b2[:, :])
        nc.sync.dma_start(w_sb[:, 0, 0:1], wt[:, 0, 0:1])
        nc.scalar.dma_start(w_sb[:, 1, 0:1], wt[:, 1, 0:1])
        for co, t0, t1 in ((0, 1, 5), (0, 5, 9), (1, 1, 5), (1, 5, 9)):
            nc.gpsimd.dma_start(w_sb[:, co, t0:t1], wt[:, co, t0:t1])

        # Pad-column zeroing: DVE tensor_copy f32 -> X_DT performs the
        # dtype conversion (memset can't write all dtypes).
        z_sb = bpool.tile([128, H, 1], F32)
        nc.vector.memset(z_sb[:], 0.0)

        ds = dpool.tile([128, RT * W], F32)
        for _ in range(N_DUMMY):
            nc.tensor.matmul(
                ds[:], d_sb[:, 0:128], d_sb[:], start=True, stop=True
            )

        # persistent x tiles for all 4 images: [n][ci]
        x_tiles = []
        for n in range(N_PER):
            row = []
            for ci in range(2):
                t = xpool.tile([128, H, W + 2], X_DT, tag=f"x{n}{ci}")
                row.append(t)
            x_tiles.append(row)

        for n in range(N_PER):
            x_sb = x_tiles[n]
            for ci in range(2):
                # pad columns for this image's tiles
                nc.vector.tensor_copy(x_sb[ci][:, :, 0:1], z_sb[:])
                nc.vector.tensor_copy(
                    x_sb[ci][:, :, W + 1 : W + 2], z_sb[:]
                )
                # contiguous load to staging in row chunks (fast DMA),
                # then VectorE restrides into the padded tile
                stg = spool.tile([128, H, W], X_DT, tag=f"s{ci}")
                eng = nc.sync if ci == 0 else nc.scalar
                for r0, r1 in XCHUNKS:
                    eng.dma_start(
                        stg[:, r0:r1],
                        xs[n, ci * 128 : (ci + 1) * 128, r0:r1, :],
                    )
                    nc.vector.tensor_copy(
                        x_sb[ci][:, r0:r1, 1 : W + 1], stg[:, r0:r1]
                    )
            if n == 0:
                # co-major for the first image: the co=0 weight half lands
                # ~12us in while its 7 row-tiles are ~24us of work, so the
                # co=1 half arrives with margin and the PE never stalls
                # (stalls here would also keep the HAM clock gate cold).
                tiles = [(rt, co) for co in range(2) for rt in range(NRT)]
            else:
                tiles = [(rt, co) for rt in range(NRT) for co in range(2)]
            for rt, co in tiles:
                h0 = rt * RT
                if True:
                    ps = ppool.tile([128, RT, W], F32)
                    for i, (kh, kw, ci) in enumerate(order):
                        dh = kh - 1
                        r0 = max(h0, -dh)
                        r1 = min(h0 + RT, H - dh)
                        lhsT = w_sb[:, co, TIDX[kh * 3 + kw], ci, :]
                        rhs = x_sb[ci][:, r0 + dh : r1 + dh, kw : kw + W]
                        nc.tensor.matmul(
                            ps[:, r0 - h0 : r1 - h0, :],
                            lhsT,
                            rhs,
                            start=(i == 0),
                            stop=(i == len(order) - 1),
                        )
                    o_sb = opool.tile([128, RT, W], X_DT)
                    nc.scalar.activation(
                        o_sb[:],
                        ps[:],
                        mybir.ActivationFunctionType.Identity,
                        bias=b_sb[:, co : co + 1],
                    )
                    oeng = nc.sync if (rt + co) % 2 == 0 else nc.gpsimd
                    oeng.dma_start(
                        out[n, co * 128 : (co + 1) * 128, h0 : h0 + RT, :],
                        o_sb[:],
                    )
    nc.compile()
    return nc


def _get_nc():
    if "nc" not in _CACHE:
        _CACHE["nc"] = _build()
    return _CACHE["nc"]


def _in_maps(x, weight, bias):
    x = np.ascontiguousarray(np.asarray(x, dtype=np.float32).astype(X_NP))
    weight = np.asarray(weight, dtype=np.float32)
    bias = np.asarray(bias, dtype=np.float32)
    # weight[co*128+o, (ci*128+c)*9 + (kh*3+kw)] -> wt[c, co, tap', ci, o]
    wt = weight.reshape(2, 128, 2, 128, 9).transpose(3, 0, 4, 2, 1)
    wt = np.ascontiguousarray(wt[:, :, WORDER].astype(W_NP))
    b2 = np.ascontiguousarray(bias.reshape(2, 128).T)
    return [
        {"xs": x[i * N_PER : (i + 1) * N_PER], "wt": wt, "b2": b2}
        for i in range(N_CORES)
    ]


def _run(x, weight, bias, trace=False):
    res = run_bass_kernel_spmd(
        _get_nc(),
        _in_maps(x, weight, bias),
        core_ids=list(range(N_CORES)),
        trace=trace,
    )
    out = np.concatenate(
        [res.results[i]["out"] for i in range(N_CORES)], axis=0
    ).astype(np.float32)
    return out, res


def kernel(x, weight, bias):
    out, _ = _run(x, weight, bias, trace=False)
    return out


def run_profiled(x, weight, bias):
    out, res = _run(x, weight, bias, trace=True)
    return out, res.exec_time_ns


# revision 9
# speedup vs baseline: 1.0178x; 1.0178x over previous
"""3x3 conv (256->256, stride 1, pad 1) as implicit GEMM on 8 TRN2 NeuronCores.

Data-parallel over batch: 32 images -> 4 per core; weight/bias replicated.

Per core, per image: x is resident in SBUF as two [128, 56, 58] channel
tiles (zero columns at w=0 and w=57 provide the horizontal conv padding,
keeping every tap a full 56-wide window). For each output row-tile of 8
rows and each of 2 output-channel tiles, 18 matmuls (9 conv taps x 2
input-channel tiles) accumulate into a PSUM tile [128, 8, 56]. Operands
are bf16 (1 cycle/row on the PE, same as fp32r, but LDWEIGHTS is 2x
faster via fast-weight-load so the stationary reload fully hides under
the ~187ns stream; fp32r's 187ns weight load was the critical path at
~210ns/matmul). Padding is handled by clipping each tap's row range via
3D access patterns; the center tap runs first with start=True so every
PSUM element's first write is an overwrite. Bias is fused into the
PSUM->SBUF eviction on the scalar engine (bf16 out, upcast on host).

Startup choreography (the PE can start ~9.5us in, so every us counts):
- Only the sync(SP)/scalar(Activation) rings have hardware DGE; gpsimd
  DMA is the slow SW path (only the tiny bias rides it). Weight pieces
  and image-0 x chunks interleave on the two fast rings in exactly the
  order the matmuls consume them.
- The weight is hosted as [c, co, tap', ci, o] with the center tap
  first (tap' order 4,0,1,2,3,5,6,7,8) and DMA'd in 3 tap pieces per co
  half; x stages in 3 row chunks. Per-slice shadow-memory dependency
  tracking lets matmuls chase individual pieces.
- Image 0 runs weight-stationary: all 7 row-tiles of co=0 accumulate in
  7 PSUM banks while taps stream in DMA-arrival order (phases below),
  so each arriving weight piece unlocks ~10x more matmul work than the
  per-tile order would. co=1 and images 1-3 use the normal per-tile
  order (everything is resident by then).
- A few dummy matmuls on zeros warm the PE clock gate (HAM) during the
  DMA wait so real matmuls run at 2.4 GHz nearly from the start.
"""

from contextlib import ExitStack

import numpy as np

import os

import concourse.bass as bass  # noqa: F401  (bass types used via tc/nc)
import concourse.tile as tile
from concourse import bacc, mybir
from concourse.bass_utils import run_bass_kernel_spmd

N_CORES = 8
N_TOTAL = 32
N_PER = N_TOTAL // N_CORES  # 4 images per core
C = 256
H = W = 56
RT = 8          # output rows per PSUM tile -> 8*56 = 448 <= 512 (one bank)
NRT = H // RT   # 7 row tiles
F32 = mybir.dt.float32
# compute dtypes for the matmul operands (storage + PE streaming format)
_X_DT_NAME = os.environ.get("CONV_X_DTYPE", "bfloat16")
_W_DT_NAME = os.environ.get("CONV_W_DTYPE", "bfloat16")
X_DT = getattr(mybir.dt, _X_DT_NAME)
W_DT = getattr(mybir.dt, _W_DT_NAME)
X_NP = mybir.dt.np(X_DT)
W_NP = mybir.dt.np(W_DT)
N_DUMMY = int(os.environ.get("CONV_N_DUMMY", "6"))

# tap order in the hosted weight: center tap first so the first DMA piece
# carries the weights the first (start=True) matmuls need
WORDER = [4, 0, 1, 2, 3, 5, 6, 7, 8]
TIDX = {t: i for i, t in enumerate(WORDER)}
# x staging row chunks: row-tile rt reads rows 8rt-1..8rt+8, so chunk
# boundaries at 9/33 cover rt0 | rt1-3 | rt4-6 cumulatively
XCHUNKS = [(0, 9), (9, 33), (33, 56)]

# image-0/co=0 weight-stationary phase schedule: (row-tiles, tap' pairs)
# emitted in DMA arrival order -- center piece, x chunk 2, tap piece 2
# (tap' 1-4), x chunk 3, tap piece 3 (tap' 5-8)
_CPAIRS = [(0, 0), (0, 1)]
_P2 = [(t, ci) for t in range(1, 5) for ci in (0, 1)]
_P3 = [(t, ci) for t in range(5, 9) for ci in (0, 1)]
PHASES = [
    (range(0, 1), _CPAIRS),          # rt0 center          (x c1, w p1)
    (range(1, 4), _CPAIRS),          # rt1-3 center        (+x c2)
    (range(0, 4), _P2),              # rt0-3 taps 1-4      (+w p2)
    (range(4, 7), _CPAIRS + _P2),    # rt4-6 center+taps   (+x c3)
    (range(0, 7), _P3),              # all rt taps 5-8     (+w p3)
]

_CACHE = {}


def _build():
    nc = bacc.Bacc(
        "TRN2", target_bir_lowering=False, debug=False, num_devices=N_CORES
    )
    xs = nc.dram_tensor(
        "xs", [N_PER, C, H, W], X_DT, kind="ExternalInput"
    ).ap()
    wt = nc.dram_tensor(
        "wt", [128, 2, 9, 2, 128], W_DT, kind="ExternalInput"
    ).ap()
    b2 = nc.dram_tensor("b2", [128, 2], F32, kind="ExternalInput").ap()
    out = nc.dram_tensor(
        "out", [N_PER, C, H, W], X_DT, kind="ExternalOutput"
    ).ap()

    # per-tile accumulation order (images 1-3 and image 0 co=1): center
    # tap (full coverage) first so its start=True write touches every
    # element of the PSUM tile; then taps in WORDER sequence.
    order = [(0, 0), (0, 1)]
    for t in range(1, 9):
        for ci in (0, 1):
            order.append((t, ci))

    def mm(ps, h0, t_idx, ci, co, x_sb, start, stop):
        tap = WORDER[t_idx]
        kh, kw = tap // 3, tap % 3
        dh = kh - 1
        r0 = max(h0, -dh)
        r1 = min(h0 + RT, H - dh)
        nc.tensor.matmul(
            ps[:, r0 - h0 : r1 - h0, :],
            w_sb[:, co, t_idx, ci, :],
            x_sb[ci][:, r0 + dh : r1 + dh, kw : kw + W],
            start=start,
            stop=stop,
        )

    def evict(ps, n, rt, co):
        o_sb = opool.tile([128, RT, W], X_DT, tag="o")
        nc.scalar.activation(
            o_sb[:],
            ps[:],
            mybir.ActivationFunctionType.Identity,
            bias=b_sb[:, co : co + 1],
        )
        oeng = nc.sync if (rt + co) % 2 == 0 else nc.scalar
        oeng.dma_start(
            out[n, co * 128 : (co + 1) * 128, rt * RT : rt * RT + RT, :],
            o_sb[:],
        )

    with tile.TileContext(nc) as tc, ExitStack() as ctx:
        wpool = ctx.enter_context(tc.tile_pool(name="w", bufs=1))
        spool = ctx.enter_context(tc.tile_pool(name="s", bufs=2))
        bpool = ctx.enter_context(tc.tile_pool(name="b", bufs=1))
        xpool = ctx.enter_context(tc.tile_pool(name="x", bufs=1))
        opool = ctx.enter_context(tc.tile_pool(name="o", bufs=4))
        ppool = ctx.enter_context(tc.tile_pool(name="p", bufs=8, space="PSUM"))

        # PE warmup: a zero tile (memset early on the gpsimd queue) feeds
        # a few dummy matmuls into a scratch PSUM bank so the HAM clock
        # gate opens to 2.4 GHz while the real weight/x DMAs land.
        d_sb = bpool.tile([128, RT * W], X_DT)
        nc.gpsimd.memset(d_sb[:], 0.0)

        w_sb = wpool.tile([128, 2, 9, 2, 128], W_DT)
        b_sb = bpool.tile([128, 2], F32)
        nc.gpsimd.dma_start(b_sb[:], b2[:, :])

        # Pad-column zeroing: DVE tensor_copy f32 -> X_DT performs the
        # dtype conversion (memset can't write all dtypes).
        z_sb = bpool.tile([128, H, 1], F32)
        nc.vector.memset(z_sb[:], 0.0)

        ds = ppool.tile([128, RT * W], F32, tag="p")
        for _ in range(N_DUMMY):
            nc.tensor.matmul(
                ds[:], d_sb[:, 0:128], d_sb[:], start=True, stop=True
            )

        # persistent x tiles for all 4 images: [n][ci]
        x_tiles = []
        for n in range(N_PER):
            row = []
            for ci in range(2):
                t = xpool.tile([128, H, W + 2], X_DT, tag=f"x{n}{ci}")
                row.append(t)
            x_tiles.append(row)

        xeng = (nc.sync, nc.scalar)

        def load_chunk(n, ci, c):
            r0, r1 = XCHUNKS[c]
            stg = stgs[ci]
            xeng[ci].dma_start(
                stg[:, r0:r1], xs[n, ci * 128 : (ci + 1) * 128, r0:r1, :]
            )
            nc.vector.tensor_copy(
                x_tiles[n][ci][:, r0:r1, 1 : W + 1], stg[:, r0:r1]
            )

        for n in range(N_PER):
            x_sb = x_tiles[n]
            for ci in range(2):
                nc.vector.tensor_copy(x_sb[ci][:, :, 0:1], z_sb[:])
                nc.vector.tensor_copy(
                    x_sb[ci][:, :, W + 1 : W + 2], z_sb[:]
                )
            stgs = []
            for ci in range(2):
                stg = spool.tile([128, H, W], X_DT, tag=f"s{ci}")
                stgs.append(stg)
            if n == 0:
                # interleave image-0 x chunks and weight pieces on the
                # two HW-DGE rings in matmul consumption order:
                #   sync:   x c1 | w co0 p1 | x c2 | w co0 p2 | x c3 | w co0 p3
                #   scalar: x c1 | w co1 p1 | x c2 | x c3 | w co1 p2 | p3
                load_chunk(0, 0, 0)
                load_chunk(0, 1, 0)
                nc.sync.dma_start(w_sb[:, 0, 0:1], wt[:, 0, 0:1])
                nc.scalar.dma_start(w_sb[:, 1, 0:1], wt[:, 1, 0:1])
                load_chunk(0, 0, 1)
                load_chunk(0, 1, 1)
                nc.sync.dma_start(w_sb[:, 0, 1:5], wt[:, 0, 1:5])
                load_chunk(0, 0, 2)
                load_chunk(0, 1, 2)
                nc.sync.dma_start(w_sb[:, 0, 5:9], wt[:, 0, 5:9])
                nc.scalar.dma_start(w_sb[:, 1, 1:5], wt[:, 1, 1:5])
                nc.scalar.dma_start(w_sb[:, 1, 5:9], wt[:, 1, 5:9])

                # co=0: weight-stationary across 7 PSUM banks, taps in
                # DMA arrival order (see PHASES); co=1: per-tile order.
                pss = []
                for rt in range(NRT):
                    ps = ppool.tile([128, RT, W], F32, tag="p")
                    pss.append(ps)
                started = set()
                for rts, pairs in PHASES:
                    for t_idx, ci in pairs:
                        for rt in rts:
                            mm(
                                pss[rt], rt * RT, t_idx, ci, 0, x_sb,
                                start=rt not in started,
                                stop=(t_idx, ci) == (8, 1),
                            )
                            started.add(rt)
                for rt in range(NRT):
                    evict(pss[rt], 0, rt, 0)
                for rt in range(NRT):
                    ps = ppool.tile([128, RT, W], F32, tag="p")
                    for i, (t_idx, ci) in enumerate(order):
                        mm(
                            ps, rt * RT, t_idx, ci, 1, x_sb,
                            start=i == 0,
                            stop=i == len(order) - 1,
                        )
                    evict(ps, 0, rt, 1)
            else:
                for c in range(3):
                    load_chunk(n, 0, c)
                    load_chunk(n, 1, c)
                for rt in range(NRT):
                    for co in range(2):
                        ps = ppool.tile([128, RT, W], F32, tag="p")
                        for i, (t_idx, ci) in enumerate(order):
                            mm(
                                ps, rt * RT, t_idx, ci, co, x_sb,
                                start=i == 0,
                                stop=i == len(order) - 1,
                            )
                        evict(ps, n, rt, co)
    nc.compile()
    return nc


def _get_nc():
    if "nc" not in _CACHE:
        _CACHE["nc"] = _build()
    return _CACHE["nc"]


def _in_maps(x, weight, bias):
    x = np.ascontiguousarray(np.asarray(x, dtype=np.float32).astype(X_NP))
    weight = np.asarray(weight, dtype=np.float32)
    bias = np.asarray(bias, dtype=np.float32)
    # weight[co*128+o, (ci*128+c)*9 + (kh*3+kw)] -> wt[c, co, tap', ci, o]
    wt = weight.reshape(2, 128, 2, 128, 9).transpose(3, 0, 4, 2, 1)
    wt = np.ascontiguousarray(wt[:, :, WORDER].astype(W_NP))
    b2 = np.ascontiguousarray(bias.reshape(2, 128).T)
    return [
        {"xs": x[i * N_PER : (i + 1) * N_PER], "wt": wt, "b2": b2}
        for i in range(N_CORES)
    ]


def _run(x, weight, bias, trace=False):
    res = run_bass_kernel_spmd(
        _get_nc(),
        _in_maps(x, weight, bias),
        core_ids=list(range(N_CORES)),
        trace=trace,
    )
    out = np.concatenate(
        [res.results[i]["out"] for i in range(N_CORES)], axis=0
    ).astype(np.float32)
    return out, res


def kernel(x, weight, bias):
    out, _ = _run(x, weight, bias, trace=False)
    return out


def run_profiled(x, weight, bias):
    out, res = _run(x, weight, bias, trace=True)
    return out, res.exec_time_ns


# revision 12
# speedup vs baseline: 1.0234x; 1.0055x over previous
"""3x3 conv (256->256, stride 1, pad 1) as implicit GEMM on 8 TRN2 NeuronCores.

Data-parallel over batch: 32 images -> 4 per core; weight/bias replicated.

Per core, per image: x is resident in SBUF as two [128, 56, 58] channel
tiles (zero columns at w=0 and w=57 provide the horizontal conv padding,
keeping every tap a full 56-wide window). For each output row-tile of 8
rows and each of 2 output-channel tiles, 18 matmuls (9 conv taps x 2
input-channel tiles) accumulate into a PSUM tile [128, 8, 56]. Operands
are bf16 (1 cycle/row on the PE, same as fp32r, but LDWEIGHTS is 2x
faster via fast-weight-load so the stationary reload fully hides under
the ~187ns stream; fp32r's 187ns weight load was the critical path at
~210ns/matmul). Padding is handled by clipping each tap's row range via
3D access patterns; the center tap runs first with start=True so every
PSUM element's first write is an overwrite. Bias is fused into the
PSUM->SBUF eviction on the scalar engine (bf16 out, upcast on host).

Startup choreography (the PE can start ~9.5us in, so every us counts):
- Only the sync(SP)/scalar(Activation) rings have hardware DGE; gpsimd
  DMA is the slow SW path (only the tiny bias rides it). Weight pieces
  and image-0 x chunks interleave on the two fast rings in exactly the
  order the matmuls consume them.
- The weight is hosted as [c, co, tap', ci, o] with the center tap
  first (tap' order 4,0,1,2,3,5,6,7,8) and DMA'd in 3 tap pieces per co
  half; x stages in 3 row chunks. Per-slice shadow-memory dependency
  tracking lets matmuls chase individual pieces.
- Image 0 runs weight-stationary: all 7 row-tiles of co=0 accumulate in
  7 PSUM banks while taps stream in DMA-arrival order (phases below),
  so each arriving weight piece unlocks ~10x more matmul work than the
  per-tile order would. co=1 and images 1-3 use the normal per-tile
  order (everything is resident by then).
- A few dummy matmuls on zeros warm the PE clock gate (HAM) during the
  DMA wait so real matmuls run at 2.4 GHz nearly from the start.
"""

from contextlib import ExitStack

import numpy as np

import os

import concourse.bass as bass  # noqa: F401  (bass types used via tc/nc)
import concourse.tile as tile
from concourse import bacc, mybir
from concourse.bass_utils import run_bass_kernel_spmd

N_CORES = 8
N_TOTAL = 32
N_PER = N_TOTAL // N_CORES  # 4 images per core
C = 256
H = W = 56
RT = 8          # output rows per PSUM tile -> 8*56 = 448 <= 512 (one bank)
NRT = H // RT   # 7 row tiles
F32 = mybir.dt.float32
# compute dtypes for the matmul operands (storage + PE streaming format)
_X_DT_NAME = os.environ.get("CONV_X_DTYPE", "bfloat16")
_W_DT_NAME = os.environ.get("CONV_W_DTYPE", "bfloat16")
X_DT = getattr(mybir.dt, _X_DT_NAME)
W_DT = getattr(mybir.dt, _W_DT_NAME)
X_NP = mybir.dt.np(X_DT)
W_NP = mybir.dt.np(W_DT)
N_DUMMY = int(os.environ.get("CONV_N_DUMMY", "6"))

# tap order in the hosted weight: center tap first so the first DMA piece
# carries the weights the first (start=True) matmuls need
WORDER = [4, 0, 1, 2, 3, 5, 6, 7, 8]
TIDX = {t: i for i, t in enumerate(WORDER)}
# x staging row chunks: row-tile rt reads rows 8rt-1..8rt+8, so chunk
# boundaries at 9/33 cover rt0 | rt1-3 | rt4-6 cumulatively
XCHUNKS = [(0, 9), (9, 33), (33, 56)]

# image-0/co=0 weight-stationary phase schedule: (row-tiles, tap' pairs)
# emitted in DMA arrival order -- center piece, x chunk 2, tap piece 2
# (tap' 1-4), x chunk 3, tap piece 3 (tap' 5-8)
_CPAIRS = [(0, 0), (0, 1)]
_P2 = [(t, ci) for t in range(1, 5) for ci in (0, 1)]
_P3 = [(t, ci) for t in range(5, 9) for ci in (0, 1)]
PHASES = [
    (range(0, 1), _CPAIRS),          # rt0 center          (x c1, w p1)
    (range(1, 4), _CPAIRS),          # rt1-3 center        (+x c2)
    (range(0, 4), _P2),              # rt0-3 taps 1-4      (+w p2)
    (range(4, 7), _CPAIRS + _P2),    # rt4-6 center+taps   (+x c3)
    (range(0, 7), _P3),              # all rt taps 5-8     (+w p3)
]

_CACHE = {}


def _build():
    nc = bacc.Bacc(
        "TRN2", target_bir_lowering=False, debug=False, num_devices=N_CORES
    )
    xs = nc.dram_tensor(
        "xs", [N_PER, C, H, W], X_DT, kind="ExternalInput"
    ).ap()
    wt = nc.dram_tensor(
        "wt", [128, 2, 9, 2, 128], W_DT, kind="ExternalInput"
    ).ap()
    b2 = nc.dram_tensor("b2", [128, 2], F32, kind="ExternalInput").ap()
    out = nc.dram_tensor(
        "out", [N_PER, C, H, W], X_DT, kind="ExternalOutput"
    ).ap()

    # per-tile accumulation order (images 1-3 and image 0 co=1): center
    # tap (full coverage) first so its start=True write touches every
    # element of the PSUM tile; then taps in WORDER sequence.
    order = [(0, 0), (0, 1)]
    for t in range(1, 9):
        for ci in (0, 1):
            order.append((t, ci))

    def mm(ps, h0, t_idx, ci, co, x_sb, start, stop):
        tap = WORDER[t_idx]
        kh, kw = tap // 3, tap % 3
        dh = kh - 1
        r0 = max(h0, -dh)
        r1 = min(h0 + RT, H - dh)
        nc.tensor.matmul(
            ps[:, r0 - h0 : r1 - h0, :],
            w_sb[:, co, t_idx, ci, :],
            x_sb[ci][:, r0 + dh : r1 + dh, kw : kw + W],
            start=start,
            stop=stop,
        )

    def evict(ps, n, rt, co, split=False):
        o_sb = opool.tile([128, RT, W], X_DT, tag="o")
        nc.scalar.activation(
            o_sb[:],
            ps[:],
            mybir.ActivationFunctionType.Identity,
            bias=b_sb[:, co : co + 1],
        )
        och = out[n, co * 128 : (co + 1) * 128]
        h0 = rt * RT
        if split:
            # last tile: halve the store across both rings to shorten
            # the end-of-kernel DMA drain
            hh = RT // 2
            nc.sync.dma_start(och[:, h0 : h0 + hh, :], o_sb[:, 0:hh])
            nc.scalar.dma_start(
                och[:, h0 + hh : h0 + RT, :], o_sb[:, hh:RT]
            )
        else:
            oeng = nc.sync if (rt + co) % 2 == 0 else nc.scalar
            oeng.dma_start(och[:, h0 : h0 + RT, :], o_sb[:])

    with tile.TileContext(nc) as tc, ExitStack() as ctx:
        wpool = ctx.enter_context(tc.tile_pool(name="w", bufs=1))
        spool = ctx.enter_context(tc.tile_pool(name="s", bufs=2))
        bpool = ctx.enter_context(tc.tile_pool(name="b", bufs=1))
        xpool = ctx.enter_context(tc.tile_pool(name="x", bufs=1))
        opool = ctx.enter_context(tc.tile_pool(name="o", bufs=4))
        ppool = ctx.enter_context(tc.tile_pool(name="p", bufs=8, space="PSUM"))

        # PE warmup: a zero tile (memset early on the gpsimd queue) feeds
        # a few dummy matmuls into a scratch PSUM bank so the HAM clock
        # gate opens to 2.4 GHz while the real weight/x DMAs land.
        d_sb = bpool.tile([128, RT * W], X_DT)
        nc.gpsimd.memset(d_sb[:], 0.0)

        w_sb = wpool.tile([128, 2, 9, 2, 128], W_DT)
        b_sb = bpool.tile([128, 2], F32)
        nc.gpsimd.dma_start(b_sb[:], b2[:, :])

        # Pad-column zeroing: DVE tensor_copy f32 -> X_DT performs the
        # dtype conversion (memset can't write all dtypes).
        z_sb = bpool.tile([128, H, 1], F32)
        nc.vector.memset(z_sb[:], 0.0)

        ds = ppool.tile([128, RT * W], F32, tag="p")
        for _ in range(N_DUMMY):
            nc.tensor.matmul(
                ds[:], d_sb[:, 0:128], d_sb[:], start=True, stop=True
            )

        # persistent x tiles for all 4 images: [n][ci]
        x_tiles = []
        for n in range(N_PER):
            row = []
            for ci in range(2):
                t = xpool.tile([128, H, W + 2], X_DT, tag=f"x{n}{ci}")
                row.append(t)
            x_tiles.append(row)

        xeng = (nc.sync, nc.scalar)

        def load_chunk(n, ci, c):
            r0, r1 = XCHUNKS[c]
            stg = stgs[ci]
            xeng[ci].dma_start(
                stg[:, r0:r1], xs[n, ci * 128 : (ci + 1) * 128, r0:r1, :]
            )
            nc.vector.tensor_copy(
                x_tiles[n][ci][:, r0:r1, 1 : W + 1], stg[:, r0:r1]
            )

        for n in range(N_PER):
            x_sb = x_tiles[n]
            for ci in range(2):
                nc.vector.tensor_copy(x_sb[ci][:, :, 0:1], z_sb[:])
                nc.vector.tensor_copy(
                    x_sb[ci][:, :, W + 1 : W + 2], z_sb[:]
                )
            stgs = []
            for ci in range(2):
                stg = spool.tile([128, H, W], X_DT, tag=f"s{ci}")
                stgs.append(stg)
            if n == 0:
                # interleave image-0 x chunks and weight pieces on the
                # two HW-DGE rings in matmul consumption order:
                #   sync:   x c1 | w co0 p1 | x c2 | w co0 p2 | x c3 | w co0 p3
                #   scalar: x c1 | w co1 p1 | x c2 | x c3 | w co1 p2 | p3
                load_chunk(0, 0, 0)
                load_chunk(0, 1, 0)
                nc.sync.dma_start(w_sb[:, 0, 0:1], wt[:, 0, 0:1])
                nc.scalar.dma_start(w_sb[:, 1, 0:1], wt[:, 1, 0:1])
                load_chunk(0, 0, 1)
                load_chunk(0, 1, 1)
                nc.sync.dma_start(w_sb[:, 0, 1:5], wt[:, 0, 1:5])
                load_chunk(0, 0, 2)
                load_chunk(0, 1, 2)
                nc.sync.dma_start(w_sb[:, 0, 5:9], wt[:, 0, 5:9])
                nc.scalar.dma_start(w_sb[:, 1, 1:5], wt[:, 1, 1:5])
                nc.scalar.dma_start(w_sb[:, 1, 5:9], wt[:, 1, 5:9])

                # co=0: weight-stationary across 7 PSUM banks, taps in
                # DMA arrival order (see PHASES); co=1: per-tile order.
                pss = []
                for rt in range(NRT):
                    ps = ppool.tile([128, RT, W], F32, tag="p")
                    pss.append(ps)
                started = set()
                for pi, (rts, pairs) in enumerate(PHASES):
                    # the x-chunk/weight-piece DMAs for phases 1 and 2
                    # land a few us after the previous phase drains; pad
                    # the PE queue with dummies so the HAM clock gate
                    # stays warm through the wait (idle > ~1.7us starts
                    # re-throttling toward 1.2 GHz)
                    for _ in range(10 if pi == 1 else 3 if pi == 2 else 0):
                        nc.tensor.matmul(
                            ds[:], d_sb[:, 0:128], d_sb[:],
                            start=True, stop=True,
                        )
                    for t_idx, ci in pairs:
                        for rt in rts:
                            mm(
                                pss[rt], rt * RT, t_idx, ci, 0, x_sb,
                                start=rt not in started,
                                stop=(t_idx, ci) == (8, 1),
                            )
                            started.add(rt)
                for rt in range(NRT):
                    evict(pss[rt], 0, rt, 0)
                for rt in range(NRT):
                    ps = ppool.tile([128, RT, W], F32, tag="p")
                    for i, (t_idx, ci) in enumerate(order):
                        mm(
                            ps, rt * RT, t_idx, ci, 1, x_sb,
                            start=i == 0,
                            stop=i == len(order) - 1,
                        )
                    evict(ps, 0, rt, 1)
            else:
                for c in range(3):
                    load_chunk(n, 0, c)
                    load_chunk(n, 1, c)
                for rt in range(NRT):
                    for co in range(2):
                        ps = ppool.tile([128, RT, W], F32, tag="p")
                        for i, (t_idx, ci) in enumerate(order):
                            mm(
                                ps, rt * RT, t_idx, ci, co, x_sb,
                                start=i == 0,
                                stop=i == len(order) - 1,
                            )
                        last = n == N_PER - 1 and rt == NRT - 1 and co == 1
                        evict(ps, n, rt, co, split=last)
    nc.compile()
    return nc


def _get_nc():
    if "nc" not in _CACHE:
        _CACHE["nc"] = _build()
    return _CACHE["nc"]


def _in_maps(x, weight, bias):
    x = np.ascontiguousarray(np.asarray(x, dtype=np.float32).astype(X_NP))
    weight = np.asarray(weight, dtype=np.float32)
    bias = np.asarray(bias, dtype=np.float32)
    # weight[co*128+o, (ci*128+c)*9 + (kh*3+kw)] -> wt[c, co, tap', ci, o]
    wt = weight.reshape(2, 128, 2, 128, 9).transpose(3, 0, 4, 2, 1)
    wt = np.ascontiguousarray(wt[:, :, WORDER].astype(W_NP))
    b2 = np.ascontiguousarray(bias.reshape(2, 128).T)
    return [
        {"xs": x[i * N_PER : (i + 1) * N_PER], "wt": wt, "b2": b2}
        for i in range(N_CORES)
    ]


def _run(x, weight, bias, trace=False):
    res = run_bass_kernel_spmd(
        _get_nc(),
        _in_maps(x, weight, bias),
        core_ids=list(range(N_CORES)),
        trace=trace,
    )
    out = np.concatenate(
        [res.results[i]["out"] for i in range(N_CORES)], axis=0
    ).astype(np.float32)
    return out, res


def kernel(x, weight, bias):
    out, _ = _run(x, weight, bias, trace=False)
    return out


def run_profiled(x, weight, bias):
    out, res = _run(x, weight, bias, trace=True)
    return out, res.exec_time_ns


# revision 14
# speedup vs baseline: 1.0294x; 1.0059x over previous
"""3x3 conv (256->256, stride 1, pad 1) as implicit GEMM on 8 TRN2 NeuronCores.

Data-parallel over batch: 32 images -> 4 per core; weight/bias replicated.

Per core, per image: x is resident in SBUF as two [128, 56, 58] channel
tiles (zero columns at w=0 and w=57 provide the horizontal conv padding,
keeping every tap a full 56-wide window). For each output row-tile of 8
rows and each of 2 output-channel tiles, 18 matmuls (9 conv taps x 2
input-channel tiles) accumulate into a PSUM tile [128, 8, 56]. Operands
are bf16 (1 cycle/row on the PE, same as fp32r, but LDWEIGHTS is 2x
faster via fast-weight-load so the stationary reload fully hides under
the ~187ns stream; fp32r's 187ns weight load was the critical path at
~210ns/matmul). Padding is handled by clipping each tap's row range via
3D access patterns; the center tap runs first with start=True so every
PSUM element's first write is an overwrite. Bias is fused into the
PSUM->SBUF eviction on the scalar engine (bf16 out, upcast on host).

Startup choreography (the PE can start ~9.5us in, so every us counts):
- Only the sync(SP)/scalar(Activation) rings have hardware DGE; gpsimd
  DMA is the slow SW path (only the tiny bias rides it). Weight pieces
  and image-0 x chunks interleave on the two fast rings in exactly the
  order the matmuls consume them.
- The weight is hosted as [c, co, tap', ci, o] with the center tap
  first (tap' order 4,0,1,2,3,5,6,7,8) and DMA'd in 3 tap pieces per co
  half; x stages in 3 row chunks. Per-slice shadow-memory dependency
  tracking lets matmuls chase individual pieces.
- Image 0 runs weight-stationary: all 7 row-tiles of co=0 accumulate in
  7 PSUM banks while taps stream in DMA-arrival order (phases below),
  so each arriving weight piece unlocks ~10x more matmul work than the
  per-tile order would. co=1 and images 1-3 use the normal per-tile
  order (everything is resident by then).
- A few dummy matmuls on zeros warm the PE clock gate (HAM) during the
  DMA wait so real matmuls run at 2.4 GHz nearly from the start.
"""

from contextlib import ExitStack

import numpy as np

import os

import concourse.bass as bass  # noqa: F401  (bass types used via tc/nc)
import concourse.tile as tile
from concourse import bacc, mybir
from concourse.bass_utils import run_bass_kernel_spmd

N_CORES = 8
N_TOTAL = 32
N_PER = N_TOTAL // N_CORES  # 4 images per core
C = 256
H = W = 56
RT = 8          # output rows per PSUM tile -> 8*56 = 448 <= 512 (one bank)
NRT = H // RT   # 7 row tiles
F32 = mybir.dt.float32
# compute dtypes for the matmul operands (storage + PE streaming format)
_X_DT_NAME = os.environ.get("CONV_X_DTYPE", "bfloat16")
_W_DT_NAME = os.environ.get("CONV_W_DTYPE", "bfloat16")
X_DT = getattr(mybir.dt, _X_DT_NAME)
W_DT = getattr(mybir.dt, _W_DT_NAME)
X_NP = mybir.dt.np(X_DT)
W_NP = mybir.dt.np(W_DT)
N_DUMMY = int(os.environ.get("CONV_N_DUMMY", "9"))

# tap order in the hosted weight: center tap first so the first DMA piece
# carries the weights the first (start=True) matmuls need
WORDER = [4, 0, 1, 2, 3, 5, 6, 7, 8]
TIDX = {t: i for i, t in enumerate(WORDER)}
# x staging row chunks: row-tile rt reads rows 8rt-1..8rt+8, so chunk
# boundaries at 9/33 cover rt0 | rt1-3 | rt4-6 cumulatively
XCHUNKS = [(0, 9), (9, 33), (33, 56)]

# image-0/co=0 weight-stationary phase schedule: (row-tiles, tap' pairs)
# emitted in DMA arrival order -- center piece, x chunk 2, tap piece 2
# (tap' 1-4), x chunk 3, tap piece 3 (tap' 5-8)
_CPAIRS = [(0, 0), (0, 1)]
_P2 = [(t, ci) for t in range(1, 5) for ci in (0, 1)]
_P3 = [(t, ci) for t in range(5, 9) for ci in (0, 1)]
PHASES = [
    (range(0, 1), _CPAIRS),          # rt0 center          (x c1, w p1)
    (range(1, 4), _CPAIRS),          # rt1-3 center        (+x c2)
    (range(0, 4), _P2),              # rt0-3 taps 1-4      (+w p2)
    (range(4, 7), _CPAIRS + _P2),    # rt4-6 center+taps   (+x c3)
    (range(0, 7), _P3),              # all rt taps 5-8     (+w p3)
]

_CACHE = {}


def _build():
    nc = bacc.Bacc(
        "TRN2", target_bir_lowering=False, debug=False, num_devices=N_CORES
    )
    xs = nc.dram_tensor(
        "xs", [N_PER, C, H, W], X_DT, kind="ExternalInput"
    ).ap()
    wt = nc.dram_tensor(
        "wt", [128, 2, 9, 2, 128], W_DT, kind="ExternalInput"
    ).ap()
    b2 = nc.dram_tensor("b2", [128, 2], F32, kind="ExternalInput").ap()
    out = nc.dram_tensor(
        "out", [N_PER, C, H, W], X_DT, kind="ExternalOutput"
    ).ap()

    # per-tile accumulation order (images 1-3 and image 0 co=1): center
    # tap (full coverage) first so its start=True write touches every
    # element of the PSUM tile; then taps in WORDER sequence.
    order = [(0, 0), (0, 1)]
    for t in range(1, 9):
        for ci in (0, 1):
            order.append((t, ci))

    def mm(ps, h0, t_idx, ci, co, x_sb, start, stop):
        tap = WORDER[t_idx]
        kh, kw = tap // 3, tap % 3
        dh = kh - 1
        r0 = max(h0, -dh)
        r1 = min(h0 + RT, H - dh)
        nc.tensor.matmul(
            ps[:, r0 - h0 : r1 - h0, :],
            w_sb[:, co, t_idx, ci, :],
            x_sb[ci][:, r0 + dh : r1 + dh, kw : kw + W],
            start=start,
            stop=stop,
        )

    def evict(ps, n, rt, co, split=False):
        o_sb = opool.tile([128, RT, W], X_DT, tag="o")
        nc.scalar.activation(
            o_sb[:],
            ps[:],
            mybir.ActivationFunctionType.Identity,
            bias=b_sb[:, co : co + 1],
        )
        och = out[n, co * 128 : (co + 1) * 128]
        h0 = rt * RT
        if split:
            # last tile: halve the store across both rings to shorten
            # the end-of-kernel DMA drain
            hh = RT // 2
            nc.sync.dma_start(och[:, h0 : h0 + hh, :], o_sb[:, 0:hh])
            nc.scalar.dma_start(
                och[:, h0 + hh : h0 + RT, :], o_sb[:, hh:RT]
            )
        else:
            oeng = nc.sync if (rt + co) % 2 == 0 else nc.scalar
            oeng.dma_start(och[:, h0 : h0 + RT, :], o_sb[:])

    with tile.TileContext(nc) as tc, ExitStack() as ctx:
        wpool = ctx.enter_context(tc.tile_pool(name="w", bufs=1))
        spool = ctx.enter_context(tc.tile_pool(name="s", bufs=2))
        bpool = ctx.enter_context(tc.tile_pool(name="b", bufs=1))
        xpool = ctx.enter_context(tc.tile_pool(name="x", bufs=1))
        opool = ctx.enter_context(tc.tile_pool(name="o", bufs=4))
        ppool = ctx.enter_context(tc.tile_pool(name="p", bufs=8, space="PSUM"))

        # PE warmup: a zero tile (memset early on the gpsimd queue) feeds
        # a few dummy matmuls into a scratch PSUM bank so the HAM clock
        # gate opens to 2.4 GHz while the real weight/x DMAs land.
        d_sb = bpool.tile([128, RT * W], X_DT)
        nc.gpsimd.memset(d_sb[:], 0.0)

        w_sb = wpool.tile([128, 2, 9, 2, 128], W_DT)
        b_sb = bpool.tile([128, 2], F32)
        nc.gpsimd.dma_start(b_sb[:], b2[:, :])

        # Pad-column zeroing: DVE tensor_copy f32 -> X_DT performs the
        # dtype conversion (memset can't write all dtypes).
        z_sb = bpool.tile([128, H, 1], F32)
        nc.vector.memset(z_sb[:], 0.0)

        ds = ppool.tile([128, RT * W], F32, tag="p")
        for _ in range(N_DUMMY):
            nc.tensor.matmul(
                ds[:], d_sb[:, 0:128], d_sb[:], start=True, stop=True
            )

        # persistent x tiles for all 4 images: [n][ci]
        x_tiles = []
        for n in range(N_PER):
            row = []
            for ci in range(2):
                t = xpool.tile([128, H, W + 2], X_DT, tag=f"x{n}{ci}")
                row.append(t)
            x_tiles.append(row)

        xeng = (nc.sync, nc.scalar)

        def load_chunk(n, ci, c):
            r0, r1 = XCHUNKS[c]
            stg = stgs[ci]
            xeng[ci].dma_start(
                stg[:, r0:r1], xs[n, ci * 128 : (ci + 1) * 128, r0:r1, :]
            )
            nc.vector.tensor_copy(
                x_tiles[n][ci][:, r0:r1, 1 : W + 1], stg[:, r0:r1]
            )

        for n in range(N_PER):
            x_sb = x_tiles[n]
            for ci in range(2):
                nc.vector.tensor_copy(x_sb[ci][:, :, 0:1], z_sb[:])
                nc.vector.tensor_copy(
                    x_sb[ci][:, :, W + 1 : W + 2], z_sb[:]
                )
            stgs = []
            for ci in range(2):
                stg = spool.tile([128, H, W], X_DT, tag=f"s{ci}")
                stgs.append(stg)
            if n == 0:
                # interleave image-0 x chunks and weight pieces on the
                # two HW-DGE rings in matmul consumption order:
                #   sync:   x c1 | w co0 p1 | x c2 | w co0 p2 | x c3 | w co0 p3
                #   scalar: x c1 | w co1 p1 | x c2 | x c3 | w co1 p2 | p3
                load_chunk(0, 0, 0)
                load_chunk(0, 1, 0)
                nc.sync.dma_start(w_sb[:, 0, 0:1], wt[:, 0, 0:1])
                nc.scalar.dma_start(w_sb[:, 1, 0:1], wt[:, 1, 0:1])
                load_chunk(0, 0, 1)
                load_chunk(0, 1, 1)
                nc.sync.dma_start(w_sb[:, 0, 1:5], wt[:, 0, 1:5])
                load_chunk(0, 0, 2)
                load_chunk(0, 1, 2)
                nc.sync.dma_start(w_sb[:, 0, 5:9], wt[:, 0, 5:9])
                nc.scalar.dma_start(w_sb[:, 1, 1:5], wt[:, 1, 1:5])
                nc.scalar.dma_start(w_sb[:, 1, 5:9], wt[:, 1, 5:9])

                # co=0: weight-stationary across 7 PSUM banks, taps in
                # DMA arrival order (see PHASES); co=1: per-tile order.
                pss = []
                for rt in range(NRT):
                    ps = ppool.tile([128, RT, W], F32, tag="p")
                    pss.append(ps)
                started = set()
                for pi, (rts, pairs) in enumerate(PHASES):
                    # the x-chunk/weight-piece DMAs for phases 1 and 2
                    # land a few us after the previous phase drains; pad
                    # the PE queue with dummies so the HAM clock gate
                    # stays warm through the wait (idle > ~1.7us starts
                    # re-throttling toward 1.2 GHz)
                    for _ in range(6 if pi == 1 else 3 if pi == 2 else 0):
                        nc.tensor.matmul(
                            ds[:], d_sb[:, 0:128], d_sb[:],
                            start=True, stop=True,
                        )
                    for t_idx, ci in pairs:
                        for rt in rts:
                            mm(
                                pss[rt], rt * RT, t_idx, ci, 0, x_sb,
                                start=rt not in started,
                                stop=(t_idx, ci) == (8, 1),
                            )
                            started.add(rt)
                for rt in range(NRT):
                    evict(pss[rt], 0, rt, 0)
                for rt in range(NRT):
                    ps = ppool.tile([128, RT, W], F32, tag="p")
                    for i, (t_idx, ci) in enumerate(order):
                        mm(
                            ps, rt * RT, t_idx, ci, 1, x_sb,
                            start=i == 0,
                            stop=i == len(order) - 1,
                        )
                    evict(ps, 0, rt, 1)
            else:
                for c in range(3):
                    load_chunk(n, 0, c)
                    load_chunk(n, 1, c)
                for rt in range(NRT):
                    for co in range(2):
                        ps = ppool.tile([128, RT, W], F32, tag="p")
                        for i, (t_idx, ci) in enumerate(order):
                            mm(
                                ps, rt * RT, t_idx, ci, co, x_sb,
                                start=i == 0,
                                stop=i == len(order) - 1,
                            )
                        last = n == N_PER - 1 and rt == NRT - 1 and co == 1
                        evict(ps, n, rt, co, split=last)
    nc.compile()
    return nc


def _get_nc():
    if "nc" not in _CACHE:
        _CACHE["nc"] = _build()
    return _CACHE["nc"]


def _in_maps(x, weight, bias):
    x = np.ascontiguousarray(np.asarray(x, dtype=np.float32).astype(X_NP))
    weight = np.asarray(weight, dtype=np.float32)
    bias = np.asarray(bias, dtype=np.float32)
    # weight[co*128+o, (ci*128+c)*9 + (kh*3+kw)] -> wt[c, co, tap', ci, o]
    wt = weight.reshape(2, 128, 2, 128, 9).transpose(3, 0, 4, 2, 1)
    wt = np.ascontiguousarray(wt[:, :, WORDER].astype(W_NP))
    b2 = np.ascontiguousarray(bias.reshape(2, 128).T)
    return [
        {"xs": x[i * N_PER : (i + 1) * N_PER], "wt": wt, "b2": b2}
        for i in range(N_CORES)
    ]


def _run(x, weight, bias, trace=False):
    res = run_bass_kernel_spmd(
        _get_nc(),
        _in_maps(x, weight, bias),
        core_ids=list(range(N_CORES)),
        trace=trace,
    )
    out = np.concatenate(
        [res.results[i]["out"] for i in range(N_CORES)], axis=0
    ).astype(np.float32)
    return out, res


def kernel(x, weight, bias):
    out, _ = _run(x, weight, bias, trace=False)
    return out


def run_profiled(x, weight, bias):
    out, res = _run(x, weight, bias, trace=True)
    return out, res.exec_time_ns
